# revision 1
# baseline (speedup 1.0000x reference)
"""3-layer GAT on 8 Trainium2 NeuronCores (Bass/Tile).

Strategy (dst-node graph partition, per sharding hint):
  - Each core owns a contiguous slice of 6250 dst nodes and all edges into them.
  - Per layer: data-parallel projection of the local node slice with an
    extended weight [W | W@al | W@ar] producing packed table rows
    [feat | ex-slot | el] (bf16) plus a local er table; AllGather replicates
    the table; per-edge rows are fetched with dma_gather (two half-tables keep
    indices within int16); attention uses exp without max-subtraction (exact
    softmax is shift-invariant; |e| <= ~2 here); per-128-edge-tile one-hot
    matmuls accumulate both the ex-weighted feature sums and the softmax
    denominators into PSUM per 128-node block; epilogue divides, adds bias,
    applies ELU and PE-transposes into the next layer's lhsT layout.
"""
import sys

import numpy as np
import ml_dtypes

try:
    from concourse import bass, mybir, tile, bacc  # noqa: F401
except ImportError:  # pragma: no cover
    sys.path.insert(0, "/opt/trn_rl_repo")
    from concourse import bass, mybir, tile, bacc  # noqa: F401
from concourse.bass_utils import run_bass_kernel_spmd

bf16 = ml_dtypes.bfloat16
f32 = np.float32

N = 50000
E = 800000
NEG = 0.2
NCORES = 8
NLOC = N // NCORES            # 6250
BLK = 128
NBLK = 49                     # ceil(6250/128)
NLOC_PAD = NBLK * BLK         # 6272
TILE = 128
HALF = 25000                  # nodes below -> table A, rest table B
HALF_ROWS = (HALF // NLOC) * NLOC_PAD   # 25088 padded rows per half-table
CH = 32                       # tiles per gather chunk

import os
DEBUG_PHASE = os.environ.get("KGAT_DEBUG", "")  # "", "proj", "gath", "nomm"

# layer configs: (in_ktiles, H, D, ROW, EXO, ELO, rhsN)
LAYERS = [
    dict(kt=2, H=4, D=32, HD=128, ROW=256, EXO=128, ELO=132, rhsN=132),
    dict(kt=1, H=4, D=32, HD=128, ROW=256, EXO=128, ELO=132, rhsN=132),
    dict(kt=1, H=1, D=64, HD=64, ROW=128, EXO=64, ELO=65, rhsN=65),
]


def _wrap_idx(vals):
    """int16 gather-index layout: element i at [i%16, i//16], replicated to
    all 8 groups of 16 partitions."""
    n = len(vals)
    assert n % 16 == 0
    arr = np.asarray(vals, np.int16).reshape(-1, 16).T  # [16, n//16]
    return np.tile(arr, (8, 1))


def _structure(src, dst):
    """Shared tile schedule + per-core index/one-hot arrays."""
    counts = np.zeros((NCORES, NBLK, 2), np.int64)
    per_core = []
    for k in range(NCORES):
        lo = k * NLOC
        m = (dst >= lo) & (dst < lo + NLOC)
        eidx = np.nonzero(m)[0]
        d_loc = dst[eidx] - lo
        half = (src[eidx] >= HALF).astype(np.int64)
        blk = d_loc // BLK
        order = np.lexsort((d_loc, blk, half))
        eidx, d_loc, half, blk = (a[order] for a in (eidx, d_loc, half, blk))
        per_core.append((eidx, d_loc, half, blk))
        np.add.at(counts[k], (blk, half), 1)
    T = np.maximum(np.ceil(counts / TILE).astype(np.int64).max(axis=0), 1)

    # shared schedule: half-major, block order; tiles per (b, h) = T[b, h]
    tile_block, tile_start, tile_stop, tile_half = [], [], [], []
    for h in range(2):
        for b in range(NBLK):
            for t in range(T[b, h]):
                tile_block.append(b)
                tile_half.append(h)
                tile_start.append(t == 0)
                tile_stop.append(t == T[b, h] - 1)
    S = len(tile_block)
    S_A = int(T[:, 0].sum())

    cores = []
    for k in range(NCORES):
        eidx, d_loc, half, blk = per_core[k]
        src_rows = np.zeros(S * TILE, np.int64)   # half-table row per slot
        oh = np.zeros((128, S * TILE), bf16)
        pos = 0
        for h in range(2):
            for b in range(NBLK):
                sel = np.nonzero((blk == b) & (half == h))[0]
                ns = len(sel)
                sl = slice(pos, pos + ns)
                s_glob = src[eidx[sel]]
                r = (s_glob // NLOC) * NLOC_PAD + s_glob % NLOC
                src_rows[sl] = r - (HALF_ROWS if h else 0)
                slots = pos + np.arange(ns)
                oh[slots % 128, (slots // 128) * 128 +
                   (d_loc[sel] - b * BLK)] = 1.0
                pos += T[b, h] * TILE
        assert src_rows.max() < 32768 and src_rows.min() >= 0
        ohT = np.ascontiguousarray(
            oh.reshape(128, S, TILE).transpose(2, 1, 0)).reshape(
                128, S * TILE)
        cores.append(dict(
            idx_src=_wrap_idx(src_rows),
            oh=oh,
            ohT=ohT,
        ))
    meta = dict(T=T, S=S, S_A=S_A,
                tile_block=tile_block, tile_start=tile_start,
                tile_stop=tile_stop)
    return meta, cores


def _chunks(t0, t1):
    out = []
    t = t0
    while t < t1:
        c = min(CH, t1 - t)
        out.append((t, c))
        t += c
    return out


def _build_program(meta):
    from concourse.masks import make_identity
    dt = mybir.dt
    S, S_A = meta["S"], meta["S_A"]
    tb, tst, tsp = meta["tile_block"], meta["tile_start"], meta["tile_stop"]

    nc = bacc.Bacc("TRN2", target_bir_lowering=False, debug=False,
                   num_devices=NCORES, num_swdge_queues=4)
    xT_in = nc.dram_tensor("xT", [128, 2 * NLOC_PAD], dt.bfloat16,
                           kind="ExternalInput")
    w_in = [nc.dram_tensor(f"W{i+1}", [128, LAYERS[i]["kt"] * (
        LAYERS[i]["HD"] + 2 * LAYERS[i]["H"])], dt.bfloat16,
        kind="ExternalInput") for i in range(3)]
    b_in = [nc.dram_tensor(f"b{i+1}", [128, LAYERS[i]["HD"]], dt.float32,
                           kind="ExternalInput") for i in range(3)]
    isrc_in = nc.dram_tensor("idx_src", [128, S * 8], dt.int16,
                             kind="ExternalInput")
    oh_in = nc.dram_tensor("oh", [128, S * TILE], dt.bfloat16,
                           kind="ExternalInput")
    ohT_in = nc.dram_tensor("ohT", [128, S * TILE], dt.bfloat16,
                            kind="ExternalInput")
    out_ext = nc.dram_tensor("out", [NLOC_PAD, 64], dt.float32,
                             kind="ExternalOutput")

    with tile.TileContext(nc) as tc:
        with (
            tc.tile_pool(name="const", bufs=1) as constp,
            tc.tile_pool(name="acts", bufs=1) as actsp,
            tc.tile_pool(name="stage", bufs=1) as stagep,
            tc.tile_pool(name="ers", bufs=1) as ersp,
            tc.tile_pool(name="stream", bufs=4) as streamp,
            tc.tile_pool(name="epi", bufs=2) as epip,
            tc.tile_pool(name="psA", bufs=2, space="PSUM") as psA,
            tc.tile_pool(name="psB", bufs=2, space="PSUM") as psB,
            tc.tile_pool(name="dram", bufs=1, space="DRAM") as dram,
        ):
            ident = constp.tile([128, 128], dt.bfloat16, tag="ident")
            make_identity(nc, ident[:])
            w_sb, b_sb = [], []
            for i, cfg in enumerate(LAYERS):
                nw = cfg["HD"] + 2 * cfg["H"]
                w = constp.tile([128, cfg["kt"], nw], dt.bfloat16,
                                tag=f"w{i}")
                nc.sync.dma_start(out=w[:], in_=w_in[i][:].rearrange(
                    "p (k c) -> p k c", k=cfg["kt"]))
                w_sb.append(w)
                bb = constp.tile([128, cfg["HD"]], dt.float32, tag=f"b{i}")
                nc.sync.dma_start(out=bb[:], in_=b_in[i][:])
                b_sb.append(bb)

            xT = actsp.tile([128, 2, NLOC_PAD], dt.bfloat16, tag="acts")
            nc.sync.dma_start(out=xT[:], in_=xT_in[:].rearrange(
                "p (k c) -> p k c", k=2))

            hT_prev = xT  # [128, kt, NLOC_PAD] layout; kt collapses via view
            for li, cfg in enumerate(LAYERS):
                H, D, HD = cfg["H"], cfg["D"], cfg["HD"]
                ROW, EXO, ELO, rhsN = (cfg[x] for x in
                                       ("ROW", "EXO", "ELO", "rhsN"))
                kt = cfg["kt"]
                last = li == 2

                tbl_loc = dram.tile([NLOC_PAD, ROW], dt.bfloat16,
                                    tag=f"tl{li}")
                tbl_full = dram.tile([NCORES * NLOC_PAD, ROW], dt.bfloat16,
                                     tag=f"tf{li}")

                # ---- projection: table rows + er table ----
                tbl_sb = stagep.tile([128, NBLK, ROW], dt.bfloat16,
                                     tag="stage")
                er_sb = ersp.tile([128, NBLK, H], dt.bfloat16, tag="ers")
                nc.vector.memset(tbl_sb[:], 0.0)
                for b in range(NBLK):
                    pp = psB.tile([128, HD + 2 * H], dt.float32, tag="proj",
                                  space="PSUM")
                    for k in range(kt):
                        if li == 0:
                            lhsT = hT_prev[:, k, b * BLK:(b + 1) * BLK]
                        else:
                            lhsT = hT_prev[:, b * BLK:(b + 1) * BLK]
                        nc.tensor.matmul(pp[:], lhsT=lhsT, rhs=w_sb[li][:, k, :],
                                         start=(k == 0), stop=(k == kt - 1))
                    nc.vector.tensor_copy(out=tbl_sb[:, b, 0:HD],
                                          in_=pp[:, 0:HD])
                    nc.vector.tensor_copy(out=tbl_sb[:, b, ELO:ELO + H],
                                          in_=pp[:, HD:HD + H])
                    nc.vector.tensor_copy(out=er_sb[:, b, 0:H],
                                          in_=pp[:, HD + H:HD + 2 * H])
                nc.sync.dma_start(
                    out=tbl_loc[:].rearrange("(b p) c -> p b c", p=128),
                    in_=tbl_sb[:])
                nc.gpsimd.collective_compute(
                    "AllGather", mybir.AluOpType.bypass,
                    replica_groups=[list(range(NCORES))],
                    ins=[tbl_loc[:].opt()], outs=[tbl_full[:].opt()])
                # dma_gather ignores AP base offsets on HW; give half B its
                # own tensor via a plain HBM->HBM copy (overlaps half-A work)
                tbl_b = dram.tile([HALF_ROWS, ROW], dt.bfloat16,
                                  tag=f"tb{li}")
                nc.sync.dma_start(out=tbl_b[:],
                                  in_=tbl_full[HALF_ROWS:2 * HALF_ROWS, :])

                if DEBUG_PHASE == "proj":
                    # projection + allgather only; dump a table slice as out
                    dbgb = stagep.tile([128, NBLK, 64], dt.bfloat16,
                                       tag="dbgb", name="dbgb")
                    nc.sync.dma_start(
                        out=dbgb[:],
                        in_=tbl_full[0:NLOC_PAD, 0:64].rearrange(
                            "(b p) c -> p b c", p=128))
                    dbgf = stagep.tile([128, NBLK, 64], dt.float32,
                                       tag="dbgf", name="dbgf")
                    nc.vector.tensor_copy(out=dbgf[:], in_=dbgb[:])
                    nc.sync.dma_start(
                        out=out_ext[:].rearrange("(b p) c -> p b c", p=128),
                        in_=dbgf[:])
                    break

                # ---- edge phase ----
                accA = stagep.tile([128, NBLK, rhsN], dt.float32, tag="stage")
                if last:
                    outacc = stagep.tile([128, NBLK, rhsN], dt.float32,
                                         tag="outacc")

                hT_new = None
                if not last:
                    hT_new = actsp.tile([128, NLOC_PAD], dt.bfloat16,
                                        tag="acts")

                cur = {"psum": None, "b": None, "half": None}
                chunk_no = [0]

                def finish_block(cur=cur, li=li, H=H, D=D, HD=HD, rhsN=rhsN,
                                 accA=accA, hT_new=hT_new, last=last):
                    ps, b, half = cur["psum"], cur["b"], cur["half"]
                    if ps is None:
                        return
                    if half == 0:
                        nc.vector.tensor_copy(out=accA[:, b, :], in_=ps[:])
                        return
                    sm = epip.tile([128, rhsN], mybir.dt.float32, tag="sm")
                    nc.vector.tensor_tensor(out=sm[:], in0=ps[:],
                                            in1=accA[:, b, :],
                                            op=mybir.AluOpType.add)
                    dr = epip.tile([128, H], mybir.dt.float32, tag="dr")
                    nc.vector.tensor_scalar_add(out=dr[:],
                                                in0=sm[:, HD:HD + H],
                                                scalar1=1e-9)
                    nc.vector.reciprocal(out=dr[:], in_=dr[:])
                    q = epip.tile([128, HD], mybir.dt.float32, tag="q")
                    nc.vector.tensor_tensor(
                        out=q[:].rearrange("p (h d) -> p h d", h=H),
                        in0=sm[:, 0:HD].rearrange("p (h d) -> p h d", h=H),
                        in1=dr[:].rearrange("p (h o) -> p h o", h=H)
                            .to_broadcast([128, H, D]),
                        op=mybir.AluOpType.mult)
                    # + bias (host-replicated to all 128 partitions)
                    nc.vector.tensor_tensor(
                        out=q[:], in0=q[:], in1=b_sb[li][:],
                        op=mybir.AluOpType.add)
                    if last:
                        nc.vector.tensor_copy(out=outacc[:, b, 0:HD],
                                              in_=q[:])
                        return
                    # elu: relu(q) + exp(min(q,0)) - 1
                    m = epip.tile([128, HD], mybir.dt.float32, tag="m")
                    nc.vector.tensor_scalar_min(out=m[:], in0=q[:],
                                                scalar1=0.0)
                    nc.scalar.activation(m[:], m[:],
                                         mybir.ActivationFunctionType.Exp)
                    hb = epip.tile([128, HD], mybir.dt.float32, tag="hb")
                    nc.vector.scalar_tensor_tensor(
                        out=hb[:], in0=q[:], scalar=0.0, in1=m[:],
                        op0=mybir.AluOpType.max, op1=mybir.AluOpType.add)
                    hbb = epip.tile([128, HD], mybir.dt.bfloat16, tag="hbb")
                    nc.vector.tensor_scalar_add(out=hbb[:], in0=hb[:],
                                                scalar1=-1.0)
                    tp = psB.tile([128, 128], mybir.dt.bfloat16, tag="tp",
                                  space="PSUM")
                    nc.tensor.transpose(tp[:], hbb[:], ident[:])
                    nc.vector.tensor_copy(
                        out=hT_new[:, b * BLK:(b + 1) * BLK], in_=tp[:])

                for (hf, t0, t1) in ((0, 0, S_A), (1, S_A, S)):
                    tblh = (tbl_full[0:HALF_ROWS, :] if hf == 0
                            else tbl_b[:])
                    for (c0, cn) in _chunks(t0, t1):
                        ni = cn * TILE
                        isb = streamp.tile([128, CH * 8], mybir.dt.int16,
                                           tag="isrc")
                        nc.sync.dma_start(out=isb[:, 0:cn * 8],
                                          in_=isrc_in[:, c0 * 8:c0 * 8 + cn * 8])
                        ohb = streamp.tile([128, CH * TILE], mybir.dt.bfloat16,
                                           tag="oh")
                        nc.sync.dma_start(
                            out=ohb[:, 0:cn * TILE],
                            in_=oh_in[:, c0 * TILE:(c0 + cn) * TILE])
                        ohTb = streamp.tile([128, CH * TILE],
                                            mybir.dt.bfloat16, tag="ohT")
                        nc.sync.dma_start(
                            out=ohTb[:, 0:cn * TILE],
                            in_=ohT_in[:, c0 * TILE:(c0 + cn) * TILE])
                        gath = streamp.tile([128, CH, ROW], mybir.dt.bfloat16,
                                            tag="gath")
                        nc.gpsimd.dma_gather(
                            out_ap=gath[:, 0:cn, :], in_ap=tblh,
                            idxs_ap=isb[:, 0:cn * 8], num_idxs=ni,
                            num_idxs_reg=ni, elem_size=ROW,
                            single_packet=False,
                            queue_num=chunk_no[0] % 4)
                        chunk_no[0] += 1
                        if DEBUG_PHASE == "gathf":
                            continue
                        # er[dst] expansion: per tile OhT.T @ er_block
                        per = psB.tile([128, CH * H], mybir.dt.float32,
                                       tag="er", name="erps", space="PSUM")
                        for t in range(cn):
                            nc.tensor.matmul(
                                per[:, t * H:(t + 1) * H],
                                lhsT=ohTb[:, t * TILE:(t + 1) * TILE],
                                rhs=er_sb[:, tb[c0 + t], 0:H],
                                start=True, stop=True)
                        est = streamp.tile([128, CH, H], mybir.dt.float32,
                                           tag="est")
                        nc.vector.tensor_tensor(
                            out=est[:, 0:cn, :],
                            in0=gath[:, 0:cn, ELO:ELO + H],
                            in1=per[:, 0:cn * H].rearrange(
                                "p (c h) -> p c h", h=H),
                            op=mybir.AluOpType.add)
                        nc.vector.scalar_tensor_tensor(
                            out=est[:, 0:cn, :], in0=est[:, 0:cn, :],
                            scalar=NEG, in1=est[:, 0:cn, :],
                            op0=mybir.AluOpType.mult,
                            op1=mybir.AluOpType.max)
                        nc.scalar.activation(
                            gath[:, 0:cn, EXO:EXO + H], est[:, 0:cn, :],
                            mybir.ActivationFunctionType.Exp)
                        nc.vector.tensor_tensor(
                            out=gath[:, 0:cn, 0:HD].rearrange(
                                "p c (h d) -> p c h d", h=H),
                            in0=gath[:, 0:cn, 0:HD].rearrange(
                                "p c (h d) -> p c h d", h=H),
                            in1=gath[:, 0:cn, EXO:EXO + H]
                                .rearrange("p c (h o) -> p c h o", h=H)
                                .to_broadcast([128, cn, H, D]),
                            op=mybir.AluOpType.mult)
                        if DEBUG_PHASE == "gath":
                            continue
                        for t in range(cn):
                            g = c0 + t
                            if tst[g]:
                                finish_block()
                                cur["psum"] = psA.tile([128, rhsN],
                                                       mybir.dt.float32,
                                                       tag="agg", name="aggp",
                                                       space="PSUM")
                                cur["b"], cur["half"] = tb[g], hf
                            nc.tensor.matmul(
                                cur["psum"][:],
                                lhsT=ohb[:, t * TILE:(t + 1) * TILE],
                                rhs=gath[:, t, 0:rhsN],
                                start=tst[g], stop=tsp[g])
                    finish_block()
                    cur["psum"] = None

                if DEBUG_PHASE.startswith("gath"):
                    dbgf = stagep.tile([128, NBLK, 64], dt.float32,
                                       tag="dbgf", name="dbgf")
                    nc.vector.memset(dbgf[:], 0.0)
                    nc.sync.dma_start(
                        out=out_ext[:].rearrange("(b p) c -> p b c", p=128),
                        in_=dbgf[:])
                    break

                if last:
                    nc.sync.dma_start(
                        out=out_ext[:].rearrange("(b p) c -> p b c", p=128),
                        in_=outacc[:, :, 0:64])
                else:
                    hT_prev = hT_new
    nc.finalize()
    return nc


def kernel(**inputs):
    x = np.asarray(inputs["x"], f32)
    src = np.asarray(inputs["src"]).astype(np.int64)
    dst = np.asarray(inputs["dst"]).astype(np.int64)

    meta, cores = _structure(src, dst)

    # host weight prep: Wext = [W | W@al_h | W@ar_h] per layer
    def wext(W, al, ar):
        W = np.asarray(W, f32)
        al = np.asarray(al, f32)
        ar = np.asarray(ar, f32)
        Hh, Dd = al.shape
        Wl = np.stack([W[:, h * Dd:(h + 1) * Dd] @ al[h] for h in range(Hh)], 1)
        Wr = np.stack([W[:, h * Dd:(h + 1) * Dd] @ ar[h] for h in range(Hh)], 1)
        return np.concatenate([W, Wl, Wr], axis=1)  # [in, HD+2H]

    wx = [wext(inputs["W1"], inputs["al1"], inputs["ar1"]),
          wext(inputs["W2"], inputs["al2"], inputs["ar2"]),
          wext(inputs["W3"], inputs["al3"], inputs["ar3"])]
    w_arrs = []
    for i, cfg in enumerate(LAYERS):
        kt, nw = cfg["kt"], cfg["HD"] + 2 * cfg["H"]
        a = np.zeros((128, kt, nw), bf16)
        for k in range(kt):
            a[:, k, :] = wx[i][k * 128:(k + 1) * 128, :].astype(bf16)
        w_arrs.append(a.reshape(128, kt * nw))
    b_arrs = [np.tile(np.asarray(inputs[f"b{i+1}"], f32).reshape(1, -1),
                      (128, 1)) for i in range(3)]

    nc = _build_program(meta)

    in_maps = []
    for k in range(NCORES):
        lo = k * NLOC
        xT = np.zeros((128, 2, NLOC_PAD), bf16)
        xs = x[lo:lo + NLOC].astype(bf16)     # [6250, 256]
        for kk in range(2):
            xT[:, kk, 0:NLOC] = xs[:, kk * 128:(kk + 1) * 128].T
        in_maps.append({
            "xT": xT.reshape(128, 2 * NLOC_PAD),
            "W1": w_arrs[0], "W2": w_arrs[1], "W3": w_arrs[2],
            "b1": b_arrs[0], "b2": b_arrs[1], "b3": b_arrs[2],
            "idx_src": cores[k]["idx_src"],
            "oh": cores[k]["oh"],
            "ohT": cores[k]["ohT"],
        })

    trace = bool(os.environ.get("KGAT_TRACE"))
    res = run_bass_kernel_spmd(nc, in_maps, core_ids=list(range(NCORES)),
                               trace=trace)
    global LAST_RESULTS
    LAST_RESULTS = res
    out = np.concatenate([res.results[k]["out"][:NLOC]
                          for k in range(NCORES)], axis=0)
    return out.astype(f32)


LAST_RESULTS = None


if __name__ == "__main__":
    import jax
    sys.path.insert(0, "/root/problem")
    import reference as ref
    with jax.default_device(jax.devices("cpu")[0]):
        inp = {k: np.asarray(v) for k, v in ref.setup_inputs().items()}
        expected = np.asarray(ref.reference(**inp))
    got = kernel(**inp)
    err = np.abs(got - expected).max()
    rel = err / np.abs(expected).max()
    print(f"abs err {err:.6f}  rel(absmax) {rel:.6f}")

